# revision 23
# baseline (speedup 1.0000x reference)
"""Multi-head attention (B=4, S=2048, H=8 heads, d_head=16) on 8 trn2 cores.

Sharding: one head per core. Per head h and batch b, a transposed-scores
dataflow with a two-engine exp split and a small-N AV matmul:

    S^T[k, q] = matmul(lhsT=Kb_dT[97,128], rhs=Q_dT[97,512])  bf16, K=97:
        rows 0-95 are the six significant hi/mid/lo cross terms of Q*K
        (matmul cost is contraction-size independent), row 96 folds the
        length-mask bias (+ones row on the Q side): st = 4*S + bias, so
        the exp stage needs no per-partition bias operand.
    P^T[k, q] = exp(st):
        q in [0,1024):   DVE tensor_scalar Schraudolph: uint16 out =
            round(st*128/ln2 + 16250.487); negative y saturates to 0
            (+0.0 in bf16), valid y bitcasts to 2^k(1+f) ~ exp (+-3%).
        q in [1024,2048): ScalarE table exp, bf16 out (+-0.4%).
    out[q, 17] += matmul(lhsT=P^T[:,128i:128(i+1)], rhs=[V|1][128,17])
        accumulated over k-tiles; 16 q-tiles/batch live in one PSUM bank
        ([128, 272] f32). Col 16 is the softmax denominator.

Cost model: PE 853(scores)+113(AV) ns/unit; DVE 1024*1.0417+125 = 1192;
Act 1024*0.8333+185+copy-amortized = 1124. ~38 units -> ~47 us vs 89.5 us
for the outT[34, S] baseline (Act-bound: 2x1024-col exp on ScalarE alone
= 2076 ns/unit). k-tiles beyond ceil(seq_len/128) are skipped (baked
per-batch at build time). Host divides num/den and reassembles heads.
"""

import ml_dtypes
import numpy as np

import concourse.bass as bass
import concourse.tile as tile
from concourse import bacc, mybir
from concourse.bass_utils import run_bass_kernel_spmd

B = 4
S = 2048
H = 8
DH = 16
KT_TILE = 128
C_SHIFT = 75.0
NEG_BIG = -1.0e30
SCHRAUD_SCALE = 128.0 / np.log(2.0)   # 184.66465
SCHRAUD_BIAS = 16256.0 - 14.0         # 127*128 - error-centering shift
F32 = mybir.dt.float32
BF16 = mybir.dt.bfloat16
U16 = mybir.dt.uint16
AVN = DH + 1  # 17: V columns + ones column (denominator)

_cache = {}


def _build(nbs):
    """Build + compile the SPMD program for per-batch k-tile counts `nbs`."""
    nb_total = sum(nbs)

    nc = bacc.Bacc(
        "TRN2",
        target_bir_lowering=False,
        debug=False,
        num_devices=8,
    )

    qT_d = nc.dram_tensor("qT", [B, 97, S], BF16, kind="ExternalInput").ap()
    kT_d = nc.dram_tensor(
        "kT", [97, nb_total * KT_TILE], BF16, kind="ExternalInput"
    ).ap()
    vo_d = nc.dram_tensor(
        "vo", [128, nb_total * AVN], BF16, kind="ExternalInput"
    ).ap()
    out_d = nc.dram_tensor(
        "outT", [B, 128, 16 * AVN], F32, kind="ExternalOutput"
    ).ap()

    with tile.TileContext(nc) as tc:
        with (
            tc.tile_pool(name="const", bufs=1) as const,
            tc.tile_pool(name="pt", bufs=4) as ptpool,
            tc.tile_pool(name="st", bufs=3, space="PSUM") as stpool,
            tc.tile_pool(name="ot", bufs=2, space="PSUM") as otpool,
            tc.tile_pool(name="ob", bufs=2) as obpool,
        ):
            q_tiles = []
            for b in range(B):
                qt = const.tile([97, S], BF16, tag=f"qT{b}")
                q_tiles.append(qt)
            kT_t = const.tile([97, nb_total * KT_TILE], BF16, tag="kT")
            vo_t = const.tile([128, nb_total * AVN], BF16, tag="vo")

            # All input DMAs go on the SP ring in strict first-use order:
            # HWDGE and the DMA transfer queue are both serial, so emission
            # order = transfer priority. Batch 0's k-tile 0 and first q
            # chunks go first in small pieces to start compute earliest.
            nc.sync.dma_start(kT_t[:, 0:128], kT_d[:, 0:128])
            nc.sync.dma_start(q_tiles[0][:, 0:512], qT_d[0][:, 0:512])
            nc.sync.dma_start(
                q_tiles[0][:, 512:2048], qT_d[0][:, 512:2048]
            )
            if nbs[0] > 1:
                nc.sync.dma_start(
                    kT_t[:, 128:nbs[0] * 128], kT_d[:, 128:nbs[0] * 128]
                )

            # Warm the PE clock gate (HAM) with dummy matmuls on zeroed
            # data during the DMA wait; the first real matmuls then run at
            # full clock. Memset on Pool (idle) so warmup starts early.
            pewarm = const.tile([97, 512], BF16, tag="pewarm")
            nc.gpsimd.memset(pewarm[:], 0.0)
            st_w = stpool.tile([128, 1024], F32, tag="st")
            for j in range(5):
                nc.tensor.matmul(
                    st_w[:, 512 * (j % 2):512 * (j % 2 + 1)],
                    pewarm[:, 0:128],
                    pewarm[:],
                    start=True,
                    stop=True,
                )
            # Prefetch the exp table set on ScalarE while input DMAs run.
            warm = const.tile([1, 1], F32, tag="warm")
            nc.vector.memset(warm[:], 0.0)
            nc.scalar.activation(
                warm[:], warm[:], mybir.ActivationFunctionType.Exp
            )

            # Remaining inputs in first-use order, all on the SP ring.
            o0, o1, o2 = (sum(nbs[:i]) * 128 for i in (1, 2, 3))
            v0, v1, v2 = (sum(nbs[:i]) * AVN for i in (1, 2, 3))
            nc.sync.dma_start(vo_t[:, 0:v0], vo_d[:, 0:v0])
            nc.sync.dma_start(kT_t[:, o0:o1], kT_d[:, o0:o1])
            nc.sync.dma_start(q_tiles[1][:], qT_d[1])
            nc.sync.dma_start(vo_t[:, v0:v1], vo_d[:, v0:v1])
            nc.sync.dma_start(kT_t[:, o1:o2], kT_d[:, o1:o2])
            nc.sync.dma_start(q_tiles[2][:], qT_d[2])
            nc.sync.dma_start(vo_t[:, v1:v2], vo_d[:, v1:v2])
            nc.sync.dma_start(
                kT_t[:, o2:nb_total * 128], kT_d[:, o2:nb_total * 128]
            )
            nc.sync.dma_start(q_tiles[3][:], qT_d[3])
            nc.sync.dma_start(vo_t[:, v2:], vo_d[:, v2:])

            # Flat unit list: one unit = one k-tile (full q range).
            units = []
            for b in range(B):
                for kt in range(nbs[b]):
                    off = sum(nbs[:b])
                    units.append((b, kt, off + kt, kt == 0, kt == nbs[b] - 1))

            pts = {}
            ots = {}

            def emit_st(u):
                b, kt, t, first, _ = units[u]
                if first:
                    # Allocate + zero this batch's AV accumulator early so
                    # the dummy matmul fills a PE pipeline gap. Exactly ONE
                    # start=True per PSUM bank: start clears the bank's
                    # has_written bits, so interleaved per-q-tile start
                    # groups would wipe each other.
                    ot_new = otpool.tile([128, 16 * AVN], F32, tag="ot")
                    ots[b] = ot_new
                    nc.tensor.matmul(
                        ot_new[:],
                        pewarm[0:1, 0:128],
                        pewarm[0:1, 0:16 * AVN],
                        start=True,
                        stop=False,
                    )
                pt = ptpool.tile([128, S], BF16, tag="pt")
                if u == 0:
                    # Unit 0 startup cut: the first two score matmuls go to
                    # SEPARATE st tiles so each 512-col exp chunk depends on
                    # only one matmul (tile deps are whole-tile).
                    for ci in range(2):
                        stz = stpool.tile([128, 1024], F32, tag="st")
                        nc.tensor.matmul(
                            stz[:, 0:512],
                            kT_t[:, t * 128:(t + 1) * 128],
                            q_tiles[b][:, 512 * ci:512 * (ci + 1)],
                            start=True,
                            stop=True,
                        )
                        nc.scalar.activation(
                            pt[:, 512 * ci:512 * (ci + 1)],
                            stz[:, 0:512],
                            mybir.ActivationFunctionType.Exp,
                        )
                    halves = [1]
                else:
                    # Last unit: emit the DVE (slower) half first so both
                    # engines' final exps overlap instead of chaining.
                    halves = [1, 0]
                for half in halves:
                    st = stpool.tile([128, 1024], F32, tag="st")
                    for j in range(2):
                        qs = 1024 * half + 512 * j
                        nc.tensor.matmul(
                            st[:, 512 * j:512 * (j + 1)],
                            kT_t[:, t * 128:(t + 1) * 128],
                            q_tiles[b][:, qs:qs + 512],
                            start=True,
                            stop=True,
                        )
                    if half == 0:
                        # ScalarE exact exp, bf16 out. Act gets the FIRST
                        # half: it finishes earlier relative to the scores
                        # that recycle its st slot (keeps PE unstalled).
                        nc.scalar.activation(
                            pt[:, 0:1024],
                            st[:],
                            mybir.ActivationFunctionType.Exp,
                        )
                    else:
                        # DVE Schraudolph exp: f32 -> uint16 (saturating,
                        # round-to-nearest), bitcast bf16.
                        nc.vector.tensor_scalar(
                            pt[:, 1024:2048].bitcast(U16),
                            st[:],
                            SCHRAUD_SCALE,
                            SCHRAUD_BIAS,
                            mybir.AluOpType.mult,
                            mybir.AluOpType.add,
                        )
                pts[u] = pt

            def emit_av(u):
                b, kt, t, first, last = units[u]
                ot = ots[b]
                pt = pts.pop(u)
                for i in range(16):
                    nc.tensor.matmul(
                        ot[:, AVN * i:AVN * (i + 1)],
                        pt[:, 128 * i:128 * (i + 1)],
                        vo_t[:, t * AVN:(t + 1) * AVN],
                        start=False,
                        stop=last,
                    )
                if last:
                    ob = obpool.tile([128, 16 * AVN], F32, tag="ob")
                    nc.scalar.copy(ob[:], ot[:])
                    nc.sync.dma_start(out_d[b], ob[:])

            for u in range(len(units)):
                emit_st(u)
                if u > 1:
                    emit_av(u - 2)
            emit_av(len(units) - 2)
            emit_av(len(units) - 1)

    nc.compile()
    return nc


def kernel(key_and_value, query, seq_len):
    key_and_value = np.asarray(key_and_value, dtype=np.float32)
    query = np.asarray(query, dtype=np.float32)
    sl = np.asarray(seq_len).reshape(-1).astype(np.int64)

    nbs = tuple(int(-(-int(s) // KT_TILE)) for s in sl)
    nb_total = sum(nbs)

    if nbs not in _cache:
        _cache[nbs] = _build(nbs)
    nc = _cache[nbs]

    k_all = key_and_value[:, :, :128]
    v_all = key_and_value[:, :, 128:]

    bf16 = ml_dtypes.bfloat16

    def himidlo(x):
        hi = x.astype(bf16)
        r = x - hi.astype(np.float32)
        mid = r.astype(bf16)
        lo = (r - mid.astype(np.float32)).astype(bf16)
        return hi, mid, lo

    # hi/mid/lo splits over the full tensors, sliced per head. The exp
    # argument must be 4*S + bias, so fold the 4x into Q (exact in f32).
    q_all_t = query.transpose(0, 2, 1) * np.float32(4.0)  # [B, 128, S]
    qhi_a, qmid_a, qlo_a = himidlo(q_all_t)
    khi_a, kmid_a, klo_a = himidlo(k_all)  # [B, S, 128]

    # Mask-bias row (head-independent): per k position, -C or -1e30.
    bias_rows = []
    for b in range(B):
        karr = np.arange(nbs[b] * 128)
        bias_rows.append(
            np.where(karr < sl[b], np.float32(-C_SHIFT), np.float32(NEG_BIG))
        )
    bias_row = np.concatenate(bias_rows).astype(bf16)  # [nb_total*128]

    in_maps = []
    for h in range(H):
        c0 = h * DH
        qT = np.empty((B, 97, S), dtype=bf16)
        for i, part in enumerate([qhi_a, qhi_a, qmid_a, qhi_a, qlo_a, qmid_a]):
            qT[:, i * DH:(i + 1) * DH] = part[:, c0:c0 + DH]
        qT[:, 96] = bf16(1.0)
        kT = np.empty((97, nb_total * 128), dtype=bf16)
        vo = np.empty((128, nb_total * AVN), dtype=bf16)
        off = 0
        for b in range(B):
            nrow = nbs[b] * 128
            sl_ = slice(off * 128, off * 128 + nrow)
            for i, part in enumerate(
                [khi_a, kmid_a, khi_a, klo_a, khi_a, kmid_a]
            ):
                kT[i * DH:(i + 1) * DH, sl_] = part[b, :nrow, c0:c0 + DH].T
            vb = v_all[b, :nrow, c0:c0 + DH].reshape(nbs[b], 128, DH)
            vo_b = np.concatenate(
                [vb.astype(bf16), np.ones((nbs[b], 128, 1), dtype=bf16)],
                axis=2,
            )  # [nb, 128, 17]
            vo[:, off * AVN:(off + nbs[b]) * AVN] = (
                vo_b.transpose(1, 0, 2).reshape(128, nbs[b] * AVN)
            )
            off += nbs[b]
        kT[96] = bias_row
        in_maps.append({
            "qT": np.ascontiguousarray(qT),
            "kT": np.ascontiguousarray(kT),
            "vo": np.ascontiguousarray(vo),
        })

    import os

    trace = bool(os.environ.get("ATTN_TRACE"))
    kw = {}
    if trace:
        kw = dict(
            trace=True,
            tmpdir=os.environ.get("ATTN_TRACE_DIR") or None,
            trace_cores=[0],
        )
    res = run_bass_kernel_spmd(nc, in_maps, core_ids=list(range(H)), **kw)
    if trace and res.exec_time_ns is not None:
        print(f"HW exec time: {res.exec_time_ns} ns")
        kernel.last_exec_time_ns = res.exec_time_ns

    out = np.empty((B, S, H * DH), dtype=np.float32)
    for h in range(H):
        o = res.results[h]["outT"].reshape(B, 128, 16, AVN).astype(np.float64)
        num = o[:, :, :, :DH]          # [B, 128p, 16i, 16]
        den = o[:, :, :, DH:]          # [B, 128p, 16i, 1]
        # q = 128*i + p  ->  order (b, i, p, d)
        out[:, :, h * DH:(h + 1) * DH] = (
            (num / den).transpose(0, 2, 1, 3).reshape(B, S, DH)
        )
    return out
